# revision 33
# baseline (speedup 1.0000x reference)
"""EntailmentConeLoss on 8 Trainium2 NeuronCores (v2).

Data-parallel over pairs (8192 pos + 32768 neg per core), prototype table
replicated in bf16 (dots of 256-dim bf16 rows carry ~0.3% error; the loss
averages 327680 energies, tolerance 2e-2).

Per core:
- Rows fetched with gpsimd dma_gather(transpose=True) on a 4-row-strided
  bf16 table view (int16 quotient indices). Pairs bucket-sorted by
  (a%4, c%4), a-residue-major, so ONE A-side gather call covers a whole
  a-residue group (4 buckets for pos, 2 for neg to bound tile size);
  C-side gathers are per bucket. 44 calls/core total (vs 64 naive) cuts
  SWDGE fixed overhead; calls round-robin 4 queues (each queue drives its
  own slice of the 16 DMA engines, so all 4 are needed).
- Elementwise a*c (DVE), a^2 (ACT Square), c^2 (alternating DVE/ACT to
  balance engines) in bf16 -- DVE runs 2-byte tensor_tensor at 2x.
- Reduction over D on the TensorEngine: per 128-pair block the elementwise
  tile is the stationary input and a ones column is moving; psum col =
  global block index, one psum tile per quantity (pp/cc/pc), accumulated
  over the two d-halves. Pair j lands at psum partition j%128, col j//128.
- ONE fused f32 epilogue over the unified [128, NPB+NNB] column space:
  cos = (pc-pp)*Rsqrt(pp*dd), octant-reduced arctan for arccos, arcsin
  series for the aperture, then a hinge relu(mrg + sgn*e) with
  host-precomputed per-column sign/margin/weight tiles that fold the
  pos/neg variants, validity masks and both means into one weighted
  reduce -> partials [128, 1]; host sums across cores.
"""
import os
os.environ.setdefault("NEURON_RT_RESET_CORES", "1")

import numpy as np

C, D = 100000, 256
P_TOT, K = 65536, 4
NCORES = 8
PPC = P_TOT // NCORES          # pos pairs per core
NPC = PPC * K                  # neg pairs per core
NBUCK = 16
EPS = np.float32(1e-6)
BETA = np.float32(0.1)
MARGIN = np.float32(0.1)
QUEUES = 4
GAT_BUFS = int(os.environ.get("KGB", "6"))
EW_BUFS = int(os.environ.get("KEB", "3"))
SQC_MODE = int(os.environ.get("KSQC", "3"))   # sqc on ACT every Nth (0=always ACT)
SQA_MODE = int(os.environ.get("KSQA", "0"))   # sqa on DVE every Nth (0=always ACT)
SINGLE_PACKET = os.environ.get("KSP", "0") == "1"
EPI_SPLIT = os.environ.get("KEPI", "1") == "2"   # 2 = pos/neg epilogue halves
INPLACE_SQ = os.environ.get("KIP", "1") == "1"   # squares overwrite gather tiles
DMA_SCRATCH = int(os.environ.get("KSCR", "16384"))  # descriptor carveout bytes

_CACHE = {}


def _build_program(caps_p, caps_n, loop_iters=1, stage=5):
    import concourse.bass as bass
    import concourse.bacc as bacc
    import concourse.mybir as mybir
    import concourse.tile as tile

    f32 = mybir.dt.float32
    bf16 = mybir.dt.bfloat16
    i16 = mybir.dt.int16
    Alu = mybir.AluOpType
    Act = mybir.ActivationFunctionType

    caps_p = list(caps_p)
    caps_n = list(caps_n)
    cum_p = np.concatenate([[0], np.cumsum(caps_p)]).astype(int)
    cum_n = np.concatenate([[0], np.cumsum(caps_n)]).astype(int)
    NPOS = int(cum_p[-1])
    NNEG = int(cum_n[-1])
    NPB = NPOS // 128
    NNB = NNEG // 128
    NCB = NPB + NNB

    HALF_PI = float(np.float32(np.pi / 2))
    PI = float(np.float32(np.pi))

    nc = bacc.Bacc("TRN2", target_bir_lowering=False, num_devices=NCORES,
                   num_swdge_queues=QUEUES, dynamic_dma_scratch_size=DMA_SCRATCH)
    tbl = nc.dram_tensor("tblbf", [C, D], bf16, kind="ExternalInput")
    posa_i = nc.dram_tensor("posa_i", [128, NPOS // 16], i16, kind="ExternalInput")
    posb_i = nc.dram_tensor("posb_i", [128, NPOS // 16], i16, kind="ExternalInput")
    nega_i = nc.dram_tensor("nega_i", [128, NNEG // 16], i16, kind="ExternalInput")
    negc_i = nc.dram_tensor("negc_i", [128, NNEG // 16], i16, kind="ExternalInput")
    wgt_d = nc.dram_tensor("wgt", [128, NCB], f32, kind="ExternalInput")
    partials = nc.dram_tensor("partials", [128, 2], f32, kind="ExternalOutput")

    with tile.TileContext(nc) as tc:
        with tc.tile_pool(name="io", bufs=1) as io, \
             tc.tile_pool(name="gat", bufs=GAT_BUFS) as gat, \
             tc.tile_pool(name="ew", bufs=EW_BUFS) as ew, \
             tc.tile_pool(name="ps", bufs=1, space="PSUM") as ps, \
             tc.tile_pool(name="tmp", bufs=1) as tmp:

            posa_t = io.tile([128, NPOS // 16], i16)
            posb_t = io.tile([128, NPOS // 16], i16)
            nega_t = io.tile([128, NNEG // 16], i16)
            negc_t = io.tile([128, NNEG // 16], i16)
            wgt_t = io.tile([128, NCB], f32)
            nc.sync.dma_start(out=posa_t[:], in_=posa_i[:])
            nc.sync.dma_start(out=posb_t[:], in_=posb_i[:])
            nc.sync.dma_start(out=nega_t[:], in_=nega_i[:])
            nc.sync.dma_start(out=negc_t[:], in_=negc_i[:])
            nc.sync.dma_start(out=wgt_t[:], in_=wgt_d[:])

            ones_t = io.tile([128, 1], bf16)
            nc.vector.memset(ones_t[:], 1.0)
            out_t = io.tile([128, 2], f32)
            nc.vector.memset(out_t[:], 0.0)

            tview = tbl[:].rearrange("(q r) d -> q r d", r=4)

            qrr = [0]
            sqc_rr = [0]

            def loop_body(_i=None):
                pp_ps = ps.tile([128, NCB], f32, tag="pp", name="pp")
                cc_ps = ps.tile([128, NCB], f32, tag="cc", name="cc")
                pc_ps = ps.tile([128, NCB], f32, tag="pc", name="pc")

                def stream(a_idx_t, c_idx_t, caps, cum, blkbase):
                    for xy in range(NBUCK):
                        cap = int(caps[xy])
                        off16 = int(cum[xy]) // 16
                        ra, rc = xy // 4, xy % 4
                        A = gat.tile([128, 2, cap], bf16, tag="ga", name="ga")
                        Cc = gat.tile([128, 2, cap], bf16, tag="gc", name="gc")
                        nc.gpsimd.dma_gather(
                            A[:], tview[:, ra, :],
                            a_idx_t[:, off16:off16 + cap // 16],
                            cap, cap, D, elem_step=4 * D, transpose=True,
                            single_packet=SINGLE_PACKET,
                            queue_num=qrr[0] % QUEUES)
                        nc.gpsimd.dma_gather(
                            Cc[:], tview[:, rc, :],
                            c_idx_t[:, off16:off16 + cap // 16],
                            cap, cap, D, elem_step=4 * D, transpose=True,
                            single_packet=SINGLE_PACKET,
                            queue_num=(qrr[0] + 1) % QUEUES)
                        qrr[0] += 2
                        if stage < 1:
                            continue
                        prod = ew.tile([128, 2, cap], bf16, tag="pr", name="pr")
                        nc.vector.tensor_tensor(
                            out=prod[:], in0=A[:], in1=Cc[:], op=Alu.mult)
                        if INPLACE_SQ:
                            # squares in place (prod already consumed A, C)
                            sqa, sqc = A, Cc
                        else:
                            sqa = ew.tile([128, 2, cap], bf16, tag="sa",
                                          name="sa")
                            sqc = ew.tile([128, 2, cap], bf16, tag="sc",
                                          name="sc")
                        sqc_rr[0] += 1
                        if SQA_MODE and sqc_rr[0] % SQA_MODE == 0:
                            nc.vector.tensor_tensor(
                                out=sqa[:], in0=A[:], in1=A[:], op=Alu.mult)
                        else:
                            nc.scalar.activation(out=sqa[:], in_=A[:],
                                                 func=Act.Square)
                        if SQC_MODE == 0 or sqc_rr[0] % SQC_MODE == 0:
                            nc.scalar.activation(out=sqc[:], in_=Cc[:],
                                                 func=Act.Square)
                        else:
                            nc.vector.tensor_tensor(
                                out=sqc[:], in0=Cc[:], in1=Cc[:], op=Alu.mult)
                        if stage < 2:
                            continue
                        for t in range(cap // 128):
                            col = blkbase + int(cum[xy]) // 128 + t
                            c0 = t * 128
                            for h, st in ((0, True), (1, False)):
                                for grp, tl in (
                                        (pp_ps, sqa[:, h, c0:c0 + 128]),
                                        (cc_ps, sqc[:, h, c0:c0 + 128]),
                                        (pc_ps, prod[:, h, c0:c0 + 128])):
                                    nc.tensor.matmul(
                                        grp[:, col:col + 1],
                                        tl, ones_t[:, 0:1],
                                        start=st, stop=not st)

                # ---------------- epilogue over a column range ----------------
                # 10 reused [128, ncol] f32 buffers (b0..b9) keep the tmp pool
                # small enough to coexist with a big DMA descriptor carveout.
                def epi(col0, col1, is_neg, out_col):
                    ncol = col1 - col0
                    b = [tmp.tile([128, ncol], f32, tag=f"ep{i}", name=f"ep{i}")
                         for i in range(10)]
                    wsl = wgt_t[:, col0:col1]
                    pp, cc, pc = b[0], b[1], b[2]
                    nc.vector.tensor_copy(pp[:], pp_ps[:, col0:col1])
                    nc.vector.tensor_copy(cc[:], cc_ps[:, col0:col1])
                    nc.vector.tensor_copy(pc[:], pc_ps[:, col0:col1])
                    if stage < 3:
                        nc.vector.tensor_tensor(out=cc[:], in0=pp[:], in1=wsl,
                                                op=Alu.mult)
                        nc.vector.tensor_reduce(
                            out=out_t[:, out_col:out_col + 1], in_=cc[:],
                            axis=mybir.AxisListType.X, op=Alu.add)
                        return
                    ppcc = b[3]
                    nc.vector.tensor_tensor(out=ppcc[:], in0=pp[:], in1=cc[:],
                                            op=Alu.add)
                    dd = b[4]
                    nc.vector.scalar_tensor_tensor(
                        out=dd[:], in0=pc[:], scalar=-2.0, in1=ppcc[:],
                        op0=Alu.mult, op1=Alu.add)
                    # dup guard: dd is rounding junk when c==p; 1.0 when valid
                    dupf = b[1]                       # cc dead
                    nc.vector.scalar_tensor_tensor(
                        out=dupf[:], in0=ppcc[:], scalar=2e-3, in1=dd[:],
                        op0=Alu.mult, op1=Alu.is_lt)
                    g = b[3]                          # ppcc dead
                    nc.vector.tensor_tensor(out=g[:], in0=pp[:], in1=dd[:],
                                            op=Alu.mult)
                    nc.vector.tensor_scalar(out=g[:], in0=g[:], scalar1=0.0,
                                            scalar2=1e-30, op0=Alu.max, op1=Alu.add)
                    # s1 = sqrt(g) + g/sqrt(g) = 2*sqrt(g) (2nd-order accurate),
                    # exactly the reference denominator 2*|p|*|diff|.
                    s0 = b[5]
                    nc.scalar.activation(out=s0[:], in_=g[:], func=Act.Sqrt)
                    r = b[6]
                    nc.vector.reciprocal(r[:], s0[:])
                    s1 = b[4]                         # dd dead
                    nc.vector.tensor_tensor(out=s1[:], in0=g[:], in1=r[:],
                                            op=Alu.mult)
                    nc.vector.tensor_tensor(out=s1[:], in0=s1[:], in1=s0[:],
                                            op=Alu.add)
                    rden = b[3]                       # g dead
                    nc.vector.reciprocal(rden[:], s1[:])
                    num = b[5]                        # s0 dead
                    nc.vector.scalar_tensor_tensor(
                        out=num[:], in0=pp[:], scalar=-1.0, in1=pc[:],
                        op0=Alu.mult, op1=Alu.add)    # pc - pp
                    cos = b[2]                        # pc dead
                    nc.vector.tensor_tensor(out=cos[:], in0=num[:], in1=rden[:],
                                            op=Alu.mult)
                    # num is really 2*(pc-pp): fold the 2 into the clamp
                    nc.vector.tensor_scalar(out=cos[:], in0=cos[:], scalar1=2.0,
                                            scalar2=float(-(1.0 - 1e-6)),
                                            op0=Alu.mult, op1=Alu.max)
                    nc.vector.tensor_scalar(out=cos[:], in0=cos[:],
                                            scalar1=float(1.0 - 1e-6),
                                            scalar2=None, op0=Alu.min)
                    nc.vector.tensor_tensor(out=cos[:], in0=cos[:], in1=dupf[:],
                                            op=Alu.mult)
                    # ang = arccos(cos) via octant-reduced arctan
                    q = b[3]                          # rden dead
                    nc.vector.tensor_tensor(out=q[:], in0=cos[:], in1=cos[:],
                                            op=Alu.mult)
                    nc.vector.tensor_scalar(out=q[:], in0=q[:], scalar1=-1.0,
                                            scalar2=1.0, op0=Alu.mult, op1=Alu.add)
                    sq = b[4]                         # s1 dead
                    nc.scalar.activation(out=sq[:], in_=q[:], func=Act.Sqrt)
                    abst = b[5]                       # num dead
                    nc.vector.tensor_scalar(out=abst[:], in0=cos[:], scalar1=-1.0,
                                            scalar2=None, op0=Alu.mult)
                    nc.vector.tensor_tensor(out=abst[:], in0=abst[:], in1=cos[:],
                                            op=Alu.max)
                    u2 = b[6]                         # r dead
                    nc.vector.tensor_tensor(out=u2[:], in0=abst[:], in1=sq[:],
                                            op=Alu.min)
                    v = b[7]
                    nc.vector.tensor_tensor(out=v[:], in0=abst[:], in1=sq[:],
                                            op=Alu.max)
                    rv = b[1]                         # dupf dead
                    nc.vector.reciprocal(rv[:], v[:])
                    rr = b[6]                         # in place over u2
                    nc.vector.tensor_tensor(out=rr[:], in0=u2[:], in1=rv[:],
                                            op=Alu.mult)
                    at = b[7]                         # v dead
                    nc.scalar.activation(out=at[:], in_=rr[:], func=Act.Arctan)
                    sgnc = b[8]
                    nc.vector.tensor_scalar(out=sgnc[:], in0=cos[:], scalar1=0.0,
                                            scalar2=None, op0=Alu.is_gt)
                    ngt = b[9]
                    nc.vector.tensor_scalar(out=ngt[:], in0=cos[:], scalar1=0.0,
                                            scalar2=None, op0=Alu.is_lt)
                    nc.vector.tensor_tensor(out=sgnc[:], in0=sgnc[:], in1=ngt[:],
                                            op=Alu.subtract)
                    big = b[5]                        # abst dead (in place)
                    nc.vector.tensor_tensor(out=big[:], in0=abst[:], in1=sq[:],
                                            op=Alu.is_gt)
                    c1 = b[4]                         # sq dead
                    nc.vector.tensor_scalar(out=c1[:], in0=big[:], scalar1=2.0,
                                            scalar2=-1.0, op0=Alu.mult, op1=Alu.add)
                    nc.vector.tensor_tensor(out=c1[:], in0=c1[:], in1=sgnc[:],
                                            op=Alu.mult)
                    w = b[9]                          # in place over ngt
                    nc.vector.tensor_scalar(out=w[:], in0=ngt[:], scalar1=PI,
                                            scalar2=-HALF_PI, op0=Alu.mult,
                                            op1=Alu.add)
                    c0 = b[9]                         # in place over w
                    nc.vector.scalar_tensor_tensor(
                        out=c0[:], in0=big[:], scalar=1.0, in1=w[:],
                        op0=Alu.mult, op1=Alu.mult)
                    nc.vector.tensor_scalar(out=c0[:], in0=c0[:], scalar1=1.0,
                                            scalar2=HALF_PI, op0=Alu.mult,
                                            op1=Alu.add)
                    ang = b[7]                        # in place over at
                    nc.vector.tensor_tensor(out=ang[:], in0=c1[:], in1=at[:],
                                            op=Alu.mult)
                    nc.vector.tensor_tensor(out=ang[:], in0=ang[:], in1=c0[:],
                                            op=Alu.add)
                    if stage < 5:
                        nc.vector.tensor_tensor(out=ang[:], in0=ang[:], in1=wsl,
                                                op=Alu.mult)
                        nc.vector.tensor_reduce(
                            out=out_t[:, out_col:out_col + 1], in_=ang[:],
                            axis=mybir.AxisListType.X, op=Alu.add)
                        return
                    # aperture = asin(beta/|p|), small-angle series. ap ~ 0.006
                    # so raw table sqrt accuracy is plenty (no Newton step).
                    sp = b[1]                         # rv dead
                    nc.scalar.activation(out=sp[:], in_=pp[:], func=Act.Sqrt)
                    y = b[2]                          # cos dead
                    nc.vector.reciprocal(y[:], sp[:])
                    nc.vector.tensor_scalar(out=y[:], in0=y[:],
                                            scalar1=float(BETA),
                                            scalar2=float(1.0 - 1e-6),
                                            op0=Alu.mult, op1=Alu.min)
                    y2 = b[1]                         # sp dead
                    nc.vector.tensor_tensor(out=y2[:], in0=y[:], in1=y[:],
                                            op=Alu.mult)
                    y3 = b[3]                         # q dead
                    nc.vector.tensor_tensor(out=y3[:], in0=y2[:], in1=y[:],
                                            op=Alu.mult)
                    ap = b[1]                         # y2 dead
                    nc.vector.scalar_tensor_tensor(
                        out=ap[:], in0=y3[:], scalar=float(1.0 / 6.0), in1=y[:],
                        op0=Alu.mult, op1=Alu.add)
                    e = b[2]                          # y dead
                    nc.vector.scalar_tensor_tensor(
                        out=e[:], in0=ap[:], scalar=-1.0, in1=ang[:],
                        op0=Alu.mult, op1=Alu.add)
                    nc.vector.tensor_scalar(out=e[:], in0=e[:], scalar1=0.0,
                                            scalar2=None, op0=Alu.max)
                    if is_neg:
                        # hinge relu(MARGIN - e)
                        nc.vector.tensor_scalar(out=e[:], in0=e[:], scalar1=-1.0,
                                                scalar2=float(MARGIN),
                                                op0=Alu.mult, op1=Alu.add)
                        nc.vector.tensor_scalar(out=e[:], in0=e[:], scalar1=0.0,
                                                scalar2=None, op0=Alu.max)
                    nc.vector.tensor_tensor(out=e[:], in0=e[:], in1=wsl,
                                            op=Alu.mult)
                    nc.vector.tensor_reduce(
                        out=out_t[:, out_col:out_col + 1], in_=e[:],
                        axis=mybir.AxisListType.X, op=Alu.add)

                stream(posa_t, posb_t, caps_p, cum_p, 0)
                if stage >= 2 and EPI_SPLIT:
                    epi(0, NPB, False, 0)
                stream(nega_t, negc_t, caps_n, cum_n, NPB)
                if stage >= 2:
                    if EPI_SPLIT:
                        epi(NPB, NCB, True, 1)
                    else:
                        epi(0, NPB, False, 0)
                        epi(NPB, NCB, True, 1)
                else:
                    nc.vector.tensor_reduce(out=out_t[:, 0:1], in_=wgt_t[:],
                                            axis=mybir.AxisListType.X, op=Alu.add)
                nc.sync.dma_start(out=partials[:], in_=out_t[:])

            if loop_iters > 1:
                with tc.For_i(0, loop_iters, 1):
                    loop_body()
            else:
                loop_body()

    nc.compile()
    return nc


def _wrap_idx(q):
    """[n] int16 -> [128, n//16] wrapped+replicated gather-index layout."""
    w = q.reshape(-1, 16).T
    return np.tile(w, (8, 1))


def _prep_stream(a_vals, c_vals, caps):
    """Bucket (a, c) pairs by (a%4, c%4); bucket xy padded to caps[xy] cols.

    Returns int16 quotient idx tiles [128, sum(caps)//16] per role and the
    validity mask [128, sum(caps)//128] in the distributed (partition=col%128,
    block=col//128) layout. Indices are wrapped per bucket (the gather-call
    granularity)."""
    cum = np.concatenate([[0], np.cumsum(caps)]).astype(int)
    total = int(cum[-1])
    key = (a_vals % 4) * 4 + (c_vals % 4)
    order = np.argsort(key, kind="stable")
    counts = np.bincount(key, minlength=NBUCK)
    a_q = np.zeros(total, np.int16)
    c_q = np.zeros(total, np.int16)
    mask = np.zeros(total, np.float32)
    off_src = 0
    for xy in range(NBUCK):
        cnt = int(counts[xy])
        assert cnt <= caps[xy], (cnt, caps[xy])
        seg = order[off_src:off_src + cnt]
        off_src += cnt
        off = int(cum[xy])
        a_q[off:off + cnt] = (a_vals[seg] // 4).astype(np.int16)
        c_q[off:off + cnt] = (c_vals[seg] // 4).astype(np.int16)
        mask[off:off + cnt] = 1.0
    a_w = np.concatenate([_wrap_idx(a_q[cum[xy]:cum[xy + 1]])
                          for xy in range(NBUCK)], axis=1)
    c_w = np.concatenate([_wrap_idx(c_q[cum[xy]:cum[xy + 1]])
                          for xy in range(NBUCK)], axis=1)
    nblk = total // 128
    mask_t = mask.reshape(nblk, 128).T.copy()
    return a_w, c_w, mask_t


def _round_cap(x):
    return max(128, ((int(x) + 127) // 128) * 128)


def _prepare(prototypes, pairs, neg_c):
    import ml_dtypes

    prototypes = np.ascontiguousarray(prototypes, dtype=np.float32)
    tblbf = prototypes.astype(ml_dtypes.bfloat16)
    pairs = np.asarray(pairs, dtype=np.int32)
    neg_c = np.asarray(neg_c, dtype=np.int32)

    shards = []
    maxp = np.zeros(NBUCK, int)
    maxn = np.zeros(NBUCK, int)
    for k in range(NCORES):
        pk = pairs[k * PPC:(k + 1) * PPC]
        nk = neg_c[k * NPC:(k + 1) * NPC]
        a, b = pk[:, 0], pk[:, 1]
        na = np.repeat(a, K)
        kp = (a % 4) * 4 + (b % 4)
        kn = (na % 4) * 4 + (nk % 4)
        maxp = np.maximum(maxp, np.bincount(kp, minlength=NBUCK))
        maxn = np.maximum(maxn, np.bincount(kn, minlength=NBUCK))
        shards.append((a, b, na, nk))
    caps_p = tuple(_round_cap(x) for x in maxp)
    caps_n = tuple(_round_cap(x) for x in maxn)
    NPB = sum(caps_p) // 128
    NNB = sum(caps_n) // 128

    in_maps = []
    for k in range(NCORES):
        a, b, na, nk = shards[k]
        pa, pb, mp = _prep_stream(a, b, caps_p)
        ng_a, ng_c, mn = _prep_stream(na, nk, caps_n)
        wgt = np.concatenate(
            [mp * np.float32(0.5 / P_TOT), mn * np.float32(0.5 / (P_TOT * K))],
            axis=1)
        in_maps.append({
            "tblbf": tblbf,
            "posa_i": pa, "posb_i": pb,
            "nega_i": ng_a, "negc_i": ng_c,
            "wgt": wgt,
        })
    return caps_p, caps_n, in_maps


def kernel(prototypes, pairs, neg_c):
    from concourse.bass_utils import run_bass_kernel_spmd

    caps_p, caps_n, in_maps = _prepare(prototypes, pairs, neg_c)
    key = (caps_p, caps_n)
    if key not in _CACHE:
        _CACHE[key] = _build_program(caps_p, caps_n)
    nc = _CACHE[key]

    res = run_bass_kernel_spmd(nc, in_maps, core_ids=list(range(NCORES)))
    tot = 0.0
    for k in range(NCORES):
        tot += float(res.results[k]["partials"].sum(dtype=np.float64))
    return np.float32(tot)


# revision 35
# speedup vs baseline: 1.0399x; 1.0399x over previous
"""EntailmentConeLoss on 8 Trainium2 NeuronCores (v2).

Data-parallel over pairs (8192 pos + 32768 neg per core), prototype table
replicated in bf16 (dots of 256-dim bf16 rows carry ~0.3% error; the loss
averages 327680 energies, tolerance 2e-2).

Per core:
- Rows fetched with gpsimd dma_gather(transpose=True) on a 4-row-strided
  bf16 table view (int16 quotient indices). Pairs bucket-sorted by
  (a%4, c%4), a-residue-major, so ONE A-side gather call covers a whole
  a-residue group (4 buckets for pos, 2 for neg to bound tile size);
  C-side gathers are per bucket. 44 calls/core total (vs 64 naive) cuts
  SWDGE fixed overhead; calls round-robin 4 queues (each queue drives its
  own slice of the 16 DMA engines, so all 4 are needed).
- Elementwise a*c (DVE), a^2 (ACT Square), c^2 (alternating DVE/ACT to
  balance engines) in bf16 -- DVE runs 2-byte tensor_tensor at 2x.
- Reduction over D on the TensorEngine: per 128-pair block the elementwise
  tile is the stationary input and a ones column is moving; psum col =
  global block index, one psum tile per quantity (pp/cc/pc), accumulated
  over the two d-halves. Pair j lands at psum partition j%128, col j//128.
- ONE fused f32 epilogue over the unified [128, NPB+NNB] column space:
  cos = (pc-pp)*Rsqrt(pp*dd), octant-reduced arctan for arccos, arcsin
  series for the aperture, then a hinge relu(mrg + sgn*e) with
  host-precomputed per-column sign/margin/weight tiles that fold the
  pos/neg variants, validity masks and both means into one weighted
  reduce -> partials [128, 1]; host sums across cores.
"""
import os
os.environ.setdefault("NEURON_RT_RESET_CORES", "1")

import numpy as np

C, D = 100000, 256
P_TOT, K = 65536, 4
NCORES = 8
PPC = P_TOT // NCORES          # pos pairs per core
NPC = PPC * K                  # neg pairs per core
NBUCK = 16
EPS = np.float32(1e-6)
BETA = np.float32(0.1)
MARGIN = np.float32(0.1)
QUEUES = 4
GAT_BUFS = int(os.environ.get("KGB", "6"))
EW_BUFS = int(os.environ.get("KEB", "3"))
SQC_MODE = int(os.environ.get("KSQC", "3"))   # sqc on ACT every Nth (0=always ACT)
SQA_MODE = int(os.environ.get("KSQA", "0"))   # sqa on DVE every Nth (0=always ACT)
SINGLE_PACKET = os.environ.get("KSP", "0") == "1"
EPI_SPLIT = os.environ.get("KEPI", "1") == "2"   # 2 = pos/neg epilogue halves
INPLACE_SQ = os.environ.get("KIP", "1") == "1"   # squares overwrite gather tiles
DMA_SCRATCH = int(os.environ.get("KSCR", "16384"))  # descriptor carveout bytes
NEG_FIRST = os.environ.get("KORD", "0") == "1"      # neg stream before pos

_CACHE = {}


def _build_program(caps_p, caps_n, loop_iters=1, stage=5):
    import concourse.bass as bass
    import concourse.bacc as bacc
    import concourse.mybir as mybir
    import concourse.tile as tile

    f32 = mybir.dt.float32
    bf16 = mybir.dt.bfloat16
    i16 = mybir.dt.int16
    Alu = mybir.AluOpType
    Act = mybir.ActivationFunctionType

    caps_p = list(caps_p)
    caps_n = list(caps_n)
    cum_p = np.concatenate([[0], np.cumsum(caps_p)]).astype(int)
    cum_n = np.concatenate([[0], np.cumsum(caps_n)]).astype(int)
    NPOS = int(cum_p[-1])
    NNEG = int(cum_n[-1])
    NPB = NPOS // 128
    NNB = NNEG // 128
    NCB = NPB + NNB

    HALF_PI = float(np.float32(np.pi / 2))
    PI = float(np.float32(np.pi))

    nc = bacc.Bacc("TRN2", target_bir_lowering=False, num_devices=NCORES,
                   num_swdge_queues=QUEUES, dynamic_dma_scratch_size=DMA_SCRATCH)
    tbl = nc.dram_tensor("tblbf", [C, D], bf16, kind="ExternalInput")
    posa_i = nc.dram_tensor("posa_i", [128, NPOS // 16], i16, kind="ExternalInput")
    posb_i = nc.dram_tensor("posb_i", [128, NPOS // 16], i16, kind="ExternalInput")
    nega_i = nc.dram_tensor("nega_i", [128, NNEG // 16], i16, kind="ExternalInput")
    negc_i = nc.dram_tensor("negc_i", [128, NNEG // 16], i16, kind="ExternalInput")
    wgt_d = nc.dram_tensor("wgt", [128, NCB], f32, kind="ExternalInput")
    partials = nc.dram_tensor("partials", [128, 2], f32, kind="ExternalOutput")

    with tile.TileContext(nc) as tc:
        with tc.tile_pool(name="io", bufs=1) as io, \
             tc.tile_pool(name="gat", bufs=GAT_BUFS) as gat, \
             tc.tile_pool(name="ew", bufs=EW_BUFS) as ew, \
             tc.tile_pool(name="ps", bufs=1, space="PSUM") as ps, \
             tc.tile_pool(name="tmp", bufs=1) as tmp:

            posa_t = io.tile([128, NPOS // 16], i16)
            posb_t = io.tile([128, NPOS // 16], i16)
            nega_t = io.tile([128, NNEG // 16], i16)
            negc_t = io.tile([128, NNEG // 16], i16)
            wgt_t = io.tile([128, NCB], f32)
            nc.sync.dma_start(out=posa_t[:], in_=posa_i[:])
            nc.sync.dma_start(out=posb_t[:], in_=posb_i[:])
            nc.sync.dma_start(out=nega_t[:], in_=nega_i[:])
            nc.sync.dma_start(out=negc_t[:], in_=negc_i[:])
            nc.sync.dma_start(out=wgt_t[:], in_=wgt_d[:])

            ones_t = io.tile([128, 1], bf16)
            nc.vector.memset(ones_t[:], 1.0)
            out_t = io.tile([128, 2], f32)
            nc.vector.memset(out_t[:], 0.0)

            tview = tbl[:].rearrange("(q r) d -> q r d", r=4)

            qrr = [0]
            sqc_rr = [0]

            def loop_body(_i=None):
                pp_ps = ps.tile([128, NCB], f32, tag="pp", name="pp")
                cc_ps = ps.tile([128, NCB], f32, tag="cc", name="cc")
                pc_ps = ps.tile([128, NCB], f32, tag="pc", name="pc")

                def stream(a_idx_t, c_idx_t, caps, cum, blkbase):
                    for xy in range(NBUCK):
                        cap = int(caps[xy])
                        off16 = int(cum[xy]) // 16
                        ra, rc = xy // 4, xy % 4
                        A = gat.tile([128, 2, cap], bf16, tag="ga", name="ga")
                        Cc = gat.tile([128, 2, cap], bf16, tag="gc", name="gc")
                        nc.gpsimd.dma_gather(
                            A[:], tview[:, ra, :],
                            a_idx_t[:, off16:off16 + cap // 16],
                            cap, cap, D, elem_step=4 * D, transpose=True,
                            single_packet=SINGLE_PACKET,
                            queue_num=qrr[0] % QUEUES)
                        nc.gpsimd.dma_gather(
                            Cc[:], tview[:, rc, :],
                            c_idx_t[:, off16:off16 + cap // 16],
                            cap, cap, D, elem_step=4 * D, transpose=True,
                            single_packet=SINGLE_PACKET,
                            queue_num=(qrr[0] + 1) % QUEUES)
                        qrr[0] += 2
                        if stage < 1:
                            continue
                        prod = ew.tile([128, 2, cap], bf16, tag="pr", name="pr")
                        nc.vector.tensor_tensor(
                            out=prod[:], in0=A[:], in1=Cc[:], op=Alu.mult)
                        if INPLACE_SQ:
                            # squares in place (prod already consumed A, C)
                            sqa, sqc = A, Cc
                        else:
                            sqa = ew.tile([128, 2, cap], bf16, tag="sa",
                                          name="sa")
                            sqc = ew.tile([128, 2, cap], bf16, tag="sc",
                                          name="sc")
                        sqc_rr[0] += 1
                        if SQA_MODE and sqc_rr[0] % SQA_MODE == 0:
                            nc.vector.tensor_tensor(
                                out=sqa[:], in0=A[:], in1=A[:], op=Alu.mult)
                        else:
                            nc.scalar.activation(out=sqa[:], in_=A[:],
                                                 func=Act.Square)
                        if SQC_MODE == 0 or sqc_rr[0] % SQC_MODE == 0:
                            nc.scalar.activation(out=sqc[:], in_=Cc[:],
                                                 func=Act.Square)
                        else:
                            nc.vector.tensor_tensor(
                                out=sqc[:], in0=Cc[:], in1=Cc[:], op=Alu.mult)
                        if stage < 2:
                            continue
                        for t in range(cap // 128):
                            col = blkbase + int(cum[xy]) // 128 + t
                            c0 = t * 128
                            for h, st in ((0, True), (1, False)):
                                for grp, tl in (
                                        (pp_ps, sqa[:, h, c0:c0 + 128]),
                                        (cc_ps, sqc[:, h, c0:c0 + 128]),
                                        (pc_ps, prod[:, h, c0:c0 + 128])):
                                    nc.tensor.matmul(
                                        grp[:, col:col + 1],
                                        tl, ones_t[:, 0:1],
                                        start=st, stop=not st)

                # ---------------- epilogue over a column range ----------------
                # 10 reused [128, ncol] f32 buffers (b0..b9) keep the tmp pool
                # small enough to coexist with a big DMA descriptor carveout.
                def epi(col0, col1, is_neg, out_col):
                    ncol = col1 - col0
                    b = [tmp.tile([128, ncol], f32, tag=f"ep{i}", name=f"ep{i}")
                         for i in range(10)]
                    wsl = wgt_t[:, col0:col1]
                    pp, cc, pc = b[0], b[1], b[2]
                    nc.vector.tensor_copy(pp[:], pp_ps[:, col0:col1])
                    nc.vector.tensor_copy(cc[:], cc_ps[:, col0:col1])
                    nc.vector.tensor_copy(pc[:], pc_ps[:, col0:col1])
                    if stage < 3:
                        nc.vector.tensor_tensor(out=cc[:], in0=pp[:], in1=wsl,
                                                op=Alu.mult)
                        nc.vector.tensor_reduce(
                            out=out_t[:, out_col:out_col + 1], in_=cc[:],
                            axis=mybir.AxisListType.X, op=Alu.add)
                        return
                    ppcc = b[3]
                    nc.vector.tensor_tensor(out=ppcc[:], in0=pp[:], in1=cc[:],
                                            op=Alu.add)
                    dd = b[4]
                    nc.vector.scalar_tensor_tensor(
                        out=dd[:], in0=pc[:], scalar=-2.0, in1=ppcc[:],
                        op0=Alu.mult, op1=Alu.add)
                    # dup guard: dd is rounding junk when c==p; 1.0 when valid
                    dupf = b[1]                       # cc dead
                    nc.vector.scalar_tensor_tensor(
                        out=dupf[:], in0=ppcc[:], scalar=2e-3, in1=dd[:],
                        op0=Alu.mult, op1=Alu.is_lt)
                    g = b[3]                          # ppcc dead
                    nc.vector.tensor_tensor(out=g[:], in0=pp[:], in1=dd[:],
                                            op=Alu.mult)
                    nc.vector.tensor_scalar(out=g[:], in0=g[:], scalar1=0.0,
                                            scalar2=1e-30, op0=Alu.max, op1=Alu.add)
                    # s1 = sqrt(g) + g/sqrt(g) = 2*sqrt(g) (2nd-order accurate),
                    # exactly the reference denominator 2*|p|*|diff|.
                    s0 = b[5]
                    nc.scalar.activation(out=s0[:], in_=g[:], func=Act.Sqrt)
                    r = b[6]
                    nc.vector.reciprocal(r[:], s0[:])
                    s1 = b[4]                         # dd dead
                    nc.vector.tensor_tensor(out=s1[:], in0=g[:], in1=r[:],
                                            op=Alu.mult)
                    nc.vector.tensor_tensor(out=s1[:], in0=s1[:], in1=s0[:],
                                            op=Alu.add)
                    rden = b[3]                       # g dead
                    nc.vector.reciprocal(rden[:], s1[:])
                    num = b[5]                        # s0 dead
                    nc.vector.scalar_tensor_tensor(
                        out=num[:], in0=pp[:], scalar=-1.0, in1=pc[:],
                        op0=Alu.mult, op1=Alu.add)    # pc - pp
                    cos = b[2]                        # pc dead
                    nc.vector.tensor_tensor(out=cos[:], in0=num[:], in1=rden[:],
                                            op=Alu.mult)
                    # num is really 2*(pc-pp): fold the 2 into the clamp
                    nc.vector.tensor_scalar(out=cos[:], in0=cos[:], scalar1=2.0,
                                            scalar2=float(-(1.0 - 1e-6)),
                                            op0=Alu.mult, op1=Alu.max)
                    nc.vector.tensor_scalar(out=cos[:], in0=cos[:],
                                            scalar1=float(1.0 - 1e-6),
                                            scalar2=None, op0=Alu.min)
                    nc.vector.tensor_tensor(out=cos[:], in0=cos[:], in1=dupf[:],
                                            op=Alu.mult)
                    # ang = arccos(cos) via octant-reduced arctan
                    q = b[3]                          # rden dead
                    nc.vector.tensor_tensor(out=q[:], in0=cos[:], in1=cos[:],
                                            op=Alu.mult)
                    nc.vector.tensor_scalar(out=q[:], in0=q[:], scalar1=-1.0,
                                            scalar2=1.0, op0=Alu.mult, op1=Alu.add)
                    sq = b[4]                         # s1 dead
                    nc.scalar.activation(out=sq[:], in_=q[:], func=Act.Sqrt)
                    abst = b[5]                       # num dead
                    nc.vector.tensor_scalar(out=abst[:], in0=cos[:], scalar1=-1.0,
                                            scalar2=None, op0=Alu.mult)
                    nc.vector.tensor_tensor(out=abst[:], in0=abst[:], in1=cos[:],
                                            op=Alu.max)
                    u2 = b[6]                         # r dead
                    nc.vector.tensor_tensor(out=u2[:], in0=abst[:], in1=sq[:],
                                            op=Alu.min)
                    v = b[7]
                    nc.vector.tensor_tensor(out=v[:], in0=abst[:], in1=sq[:],
                                            op=Alu.max)
                    rv = b[1]                         # dupf dead
                    nc.vector.reciprocal(rv[:], v[:])
                    rr = b[6]                         # in place over u2
                    nc.vector.tensor_tensor(out=rr[:], in0=u2[:], in1=rv[:],
                                            op=Alu.mult)
                    at = b[7]                         # v dead
                    nc.scalar.activation(out=at[:], in_=rr[:], func=Act.Arctan)
                    sgnc = b[8]
                    nc.vector.tensor_scalar(out=sgnc[:], in0=cos[:], scalar1=0.0,
                                            scalar2=None, op0=Alu.is_gt)
                    ngt = b[9]
                    nc.vector.tensor_scalar(out=ngt[:], in0=cos[:], scalar1=0.0,
                                            scalar2=None, op0=Alu.is_lt)
                    nc.vector.tensor_tensor(out=sgnc[:], in0=sgnc[:], in1=ngt[:],
                                            op=Alu.subtract)
                    big = b[5]                        # abst dead (in place)
                    nc.vector.tensor_tensor(out=big[:], in0=abst[:], in1=sq[:],
                                            op=Alu.is_gt)
                    c1 = b[4]                         # sq dead
                    nc.vector.tensor_scalar(out=c1[:], in0=big[:], scalar1=2.0,
                                            scalar2=-1.0, op0=Alu.mult, op1=Alu.add)
                    nc.vector.tensor_tensor(out=c1[:], in0=c1[:], in1=sgnc[:],
                                            op=Alu.mult)
                    w = b[9]                          # in place over ngt
                    nc.vector.tensor_scalar(out=w[:], in0=ngt[:], scalar1=PI,
                                            scalar2=-HALF_PI, op0=Alu.mult,
                                            op1=Alu.add)
                    c0 = b[9]                         # in place over w
                    nc.vector.scalar_tensor_tensor(
                        out=c0[:], in0=big[:], scalar=1.0, in1=w[:],
                        op0=Alu.mult, op1=Alu.mult)
                    nc.vector.tensor_scalar(out=c0[:], in0=c0[:], scalar1=1.0,
                                            scalar2=HALF_PI, op0=Alu.mult,
                                            op1=Alu.add)
                    ang = b[7]                        # in place over at
                    nc.vector.tensor_tensor(out=ang[:], in0=c1[:], in1=at[:],
                                            op=Alu.mult)
                    nc.vector.tensor_tensor(out=ang[:], in0=ang[:], in1=c0[:],
                                            op=Alu.add)
                    if stage < 5:
                        nc.vector.tensor_tensor(out=ang[:], in0=ang[:], in1=wsl,
                                                op=Alu.mult)
                        nc.vector.tensor_reduce(
                            out=out_t[:, out_col:out_col + 1], in_=ang[:],
                            axis=mybir.AxisListType.X, op=Alu.add)
                        return
                    # aperture = asin(beta/|p|), small-angle series. ap ~ 0.006
                    # so raw table sqrt accuracy is plenty (no Newton step).
                    sp = b[1]                         # rv dead
                    nc.scalar.activation(out=sp[:], in_=pp[:], func=Act.Sqrt)
                    y = b[2]                          # cos dead
                    nc.vector.reciprocal(y[:], sp[:])
                    nc.vector.tensor_scalar(out=y[:], in0=y[:],
                                            scalar1=float(BETA),
                                            scalar2=float(1.0 - 1e-6),
                                            op0=Alu.mult, op1=Alu.min)
                    y2 = b[1]                         # sp dead
                    nc.vector.tensor_tensor(out=y2[:], in0=y[:], in1=y[:],
                                            op=Alu.mult)
                    y3 = b[3]                         # q dead
                    nc.vector.tensor_tensor(out=y3[:], in0=y2[:], in1=y[:],
                                            op=Alu.mult)
                    ap = b[1]                         # y2 dead
                    nc.vector.scalar_tensor_tensor(
                        out=ap[:], in0=y3[:], scalar=float(1.0 / 6.0), in1=y[:],
                        op0=Alu.mult, op1=Alu.add)
                    e = b[2]                          # y dead
                    nc.vector.scalar_tensor_tensor(
                        out=e[:], in0=ap[:], scalar=-1.0, in1=ang[:],
                        op0=Alu.mult, op1=Alu.add)
                    nc.vector.tensor_scalar(out=e[:], in0=e[:], scalar1=0.0,
                                            scalar2=None, op0=Alu.max)
                    if is_neg:
                        # hinge relu(MARGIN - e)
                        nc.vector.tensor_scalar(out=e[:], in0=e[:], scalar1=-1.0,
                                                scalar2=float(MARGIN),
                                                op0=Alu.mult, op1=Alu.add)
                        nc.vector.tensor_scalar(out=e[:], in0=e[:], scalar1=0.0,
                                                scalar2=None, op0=Alu.max)
                    nc.vector.tensor_tensor(out=e[:], in0=e[:], in1=wsl,
                                            op=Alu.mult)
                    nc.vector.tensor_reduce(
                        out=out_t[:, out_col:out_col + 1], in_=e[:],
                        axis=mybir.AxisListType.X, op=Alu.add)

                if NEG_FIRST:
                    stream(nega_t, negc_t, caps_n, cum_n, NPB)
                    stream(posa_t, posb_t, caps_p, cum_p, 0)
                    if stage >= 2:
                        epi(NPB, NCB, True, 1)
                        epi(0, NPB, False, 0)
                else:
                    stream(posa_t, posb_t, caps_p, cum_p, 0)
                    if stage >= 2 and EPI_SPLIT:
                        epi(0, NPB, False, 0)
                    stream(nega_t, negc_t, caps_n, cum_n, NPB)
                    if stage >= 2:
                        if EPI_SPLIT:
                            epi(NPB, NCB, True, 1)
                        else:
                            epi(0, NPB, False, 0)
                            epi(NPB, NCB, True, 1)
                if stage < 2:
                    nc.vector.tensor_reduce(out=out_t[:, 0:1], in_=wgt_t[:],
                                            axis=mybir.AxisListType.X, op=Alu.add)
                nc.sync.dma_start(out=partials[:], in_=out_t[:])

            if loop_iters > 1:
                with tc.For_i(0, loop_iters, 1):
                    loop_body()
            else:
                loop_body()

    nc.compile()
    return nc


def _wrap_idx(q):
    """[n] int16 -> [128, n//16] wrapped+replicated gather-index layout."""
    w = q.reshape(-1, 16).T
    return np.tile(w, (8, 1))


def _prep_stream(a_vals, c_vals, caps):
    """Bucket (a, c) pairs by (a%4, c%4); bucket xy padded to caps[xy] cols.

    Returns int16 quotient idx tiles [128, sum(caps)//16] per role and the
    validity mask [128, sum(caps)//128] in the distributed (partition=col%128,
    block=col//128) layout. Indices are wrapped per bucket (the gather-call
    granularity)."""
    cum = np.concatenate([[0], np.cumsum(caps)]).astype(int)
    total = int(cum[-1])
    key = (a_vals % 4) * 4 + (c_vals % 4)
    order = np.argsort(key, kind="stable")
    counts = np.bincount(key, minlength=NBUCK)
    a_q = np.zeros(total, np.int16)
    c_q = np.zeros(total, np.int16)
    mask = np.zeros(total, np.float32)
    off_src = 0
    for xy in range(NBUCK):
        cnt = int(counts[xy])
        assert cnt <= caps[xy], (cnt, caps[xy])
        seg = order[off_src:off_src + cnt]
        off_src += cnt
        off = int(cum[xy])
        a_q[off:off + cnt] = (a_vals[seg] // 4).astype(np.int16)
        c_q[off:off + cnt] = (c_vals[seg] // 4).astype(np.int16)
        mask[off:off + cnt] = 1.0
    a_w = np.concatenate([_wrap_idx(a_q[cum[xy]:cum[xy + 1]])
                          for xy in range(NBUCK)], axis=1)
    c_w = np.concatenate([_wrap_idx(c_q[cum[xy]:cum[xy + 1]])
                          for xy in range(NBUCK)], axis=1)
    nblk = total // 128
    mask_t = mask.reshape(nblk, 128).T.copy()
    return a_w, c_w, mask_t


def _round_cap(x):
    return max(128, ((int(x) + 127) // 128) * 128)


def _prepare(prototypes, pairs, neg_c):
    import ml_dtypes

    prototypes = np.ascontiguousarray(prototypes, dtype=np.float32)
    tblbf = prototypes.astype(ml_dtypes.bfloat16)
    pairs = np.asarray(pairs, dtype=np.int32)
    neg_c = np.asarray(neg_c, dtype=np.int32)

    shards = []
    maxp = np.zeros(NBUCK, int)
    maxn = np.zeros(NBUCK, int)
    for k in range(NCORES):
        pk = pairs[k * PPC:(k + 1) * PPC]
        nk = neg_c[k * NPC:(k + 1) * NPC]
        a, b = pk[:, 0], pk[:, 1]
        na = np.repeat(a, K)
        kp = (a % 4) * 4 + (b % 4)
        kn = (na % 4) * 4 + (nk % 4)
        maxp = np.maximum(maxp, np.bincount(kp, minlength=NBUCK))
        maxn = np.maximum(maxn, np.bincount(kn, minlength=NBUCK))
        shards.append((a, b, na, nk))
    caps_p = tuple(_round_cap(x) for x in maxp)
    caps_n = tuple(_round_cap(x) for x in maxn)
    NPB = sum(caps_p) // 128
    NNB = sum(caps_n) // 128

    in_maps = []
    for k in range(NCORES):
        a, b, na, nk = shards[k]
        pa, pb, mp = _prep_stream(a, b, caps_p)
        ng_a, ng_c, mn = _prep_stream(na, nk, caps_n)
        wgt = np.concatenate(
            [mp * np.float32(0.5 / P_TOT), mn * np.float32(0.5 / (P_TOT * K))],
            axis=1)
        in_maps.append({
            "tblbf": tblbf,
            "posa_i": pa, "posb_i": pb,
            "nega_i": ng_a, "negc_i": ng_c,
            "wgt": wgt,
        })
    return caps_p, caps_n, in_maps


def kernel(prototypes, pairs, neg_c):
    from concourse.bass_utils import run_bass_kernel_spmd

    caps_p, caps_n, in_maps = _prepare(prototypes, pairs, neg_c)
    key = (caps_p, caps_n)
    if key not in _CACHE:
        _CACHE[key] = _build_program(caps_p, caps_n)
    nc = _CACHE[key]

    res = run_bass_kernel_spmd(nc, in_maps, core_ids=list(range(NCORES)))
    tot = 0.0
    for k in range(NCORES):
        tot += float(res.results[k]["partials"].sum(dtype=np.float64))
    return np.float32(tot)
